# revision 17
# baseline (speedup 1.0000x reference)
"""CenterLoss (gather + MSE mean) on 8 Trainium2 NeuronCores.

Strategy (data-parallel + label-sort, per sharding hint):
  Expand  sum(x-c_l)^2 = sum x^2 - 2 sum x_i.c_{l_i} + sum_l n_l |c_l|^2
  and sort rows by label on the host (a legal data-parallel resharding:
  the mean is order-invariant). After the global sort each 1024-row
  chunk touches only ~9 distinct classes (<=CH_CLS=32 with huge margin),
  so the cross/center terms need only a 32-row table window per chunk:

  - Shard the sorted x / labels along N across 8 cores.
  - Per core, stream x in [128, 8, 512] f32 chunks (16 chunks of 2MB).
    ACT squares each chunk in place with a row-accumulate -> sum x^2.
  - PE computes per-class partial sums S = A^T @ x per chunk: 8 bf16
    matmuls (lhsT = host-built one-hot A [128, 32], rhs = x cast to
    bf16 by DVE) accumulating in one PSUM bank. One-hot weights are
    exact in bf16; the x rounding only perturbs the small cross term
    (~3e-7 of the loss). fp32r would avoid the cast but the BIR
    verifier rejects DMA-produced fp32r operands.
  - gpsimd.dma_gather pulls the chunk's <=32 distinct center rows
    (f32, 2KB each, 64KB/chunk) from the DRAM table; DVE contracts
    -2*sum(S.c) in one tensor_tensor_reduce; ACT squares the window
    rows (accum -> |c|^2 per class), which the epilogue multiplies by
    host-side per-chunk class counts.
  - Epilogue: fold the three terms into one [128, T+2] column tile,
    free-dim reduce, ones-matmul partition reduce -> per-core scalar.
    Host sums the 8 partials and divides by N*FEAT.

  vs. the direct gather+subtract kernel this removes the 8MB/core
  center-gather HBM traffic (~20% of total) and the 70us DVE subtract;
  HBM traffic is ~35MB/core, within ~15% of the streaming floor.
"""
import numpy as np
from contextlib import ExitStack

import ml_dtypes

import concourse.tile as tile
from concourse import bacc, mybir, bass_isa
from concourse.bass_utils import run_bass_kernel_spmd

N, FEAT, NCLASS = 131072, 512, 1000
NCORES = 8
SHARD = N // NCORES          # 16384 rows per core
CHUNK = 1024                 # rows per pipeline chunk
T = SHARD // CHUNK           # 16 chunks
ROWS_P = CHUNK // 128        # 8 rows per partition per chunk
CH_CLS = 32                  # center-window slots per chunk (~9 used)

TRACE = False                # set by test.py for profiled runs
LAST_RESULTS = None          # BassKernelResults of the last kernel() call


def _build_nc():
    nc = bacc.Bacc("TRN2", target_bir_lowering=False, debug=False,
                   enable_asserts=False, num_swdge_queues=4)
    x = nc.dram_tensor("x", [SHARD, FEAT], mybir.dt.float32,
                       kind="ExternalInput")
    a = nc.dram_tensor("a", [128, T * ROWS_P * CH_CLS], mybir.dt.bfloat16,
                       kind="ExternalInput")
    idxs = nc.dram_tensor("idxs", [128, T * CH_CLS // 16], mybir.dt.int16,
                          kind="ExternalInput")
    cnts = nc.dram_tensor("cnts", [CH_CLS, T], mybir.dt.float32,
                          kind="ExternalInput")
    tbl = nc.dram_tensor("tbl", [NCLASS, FEAT], mybir.dt.float32,
                         kind="ExternalInput")
    out = nc.dram_tensor("out", [1, 1], mybir.dt.float32,
                         kind="ExternalOutput")

    with tile.TileContext(nc) as tc, ExitStack() as ctx:
        xp = ctx.enter_context(tc.tile_pool(name="xp", bufs=5))
        xbp = ctx.enter_context(tc.tile_pool(name="xbp", bufs=4))
        cp = ctx.enter_context(tc.tile_pool(name="cp", bufs=4))
        scp = ctx.enter_context(tc.tile_pool(name="scp", bufs=4))
        sp = ctx.enter_context(tc.tile_pool(name="small", bufs=1))
        pp = ctx.enter_context(tc.tile_pool(name="pp", bufs=4, space="PSUM"))

        # Small inputs ride the scalar engine's HWDGE ring so chunk 0's
        # gather + matmul start within ~2us, ahead of the x megabytes.
        idxt = sp.tile([128, T * CH_CLS // 16], mybir.dt.int16)
        nc.scalar.dma_start(idxt[:], idxs.ap())
        cntt = sp.tile([CH_CLS, T], mybir.dt.float32)
        nc.scalar.dma_start(cntt[:], cnts.ap())
        # One-hot A in 4 pieces so chunk 0 waits on 512KB, not 2MB.
        APC = 4
        acols = T * ROWS_P * CH_CLS // APC
        a_sb = sp.tile([128, T * ROWS_P * CH_CLS], mybir.dt.bfloat16)
        for i in range(APC):
            nc.scalar.dma_start(a_sb[:, i * acols:(i + 1) * acols],
                                a.ap()[:, i * acols:(i + 1) * acols])

        # acc columns: [0:T] = per-chunk sum x^2 (all 128 rows),
        # col T = sum_l n_l|c_l|^2, col T+1 = -2 sum S.c (rows 0:32).
        acc = sp.tile([128, T + 2], mybir.dt.float32)
        nc.vector.memset(acc[:], 0.0)
        crossc = sp.tile([CH_CLS, T], mybir.dt.float32)   # -2*S.c per chunk
        csq = sp.tile([CH_CLS, T], mybir.dt.float32)      # |c_s|^2 per chunk

        xr = x.ap().rearrange("(t p u) f -> t p u f", t=T, p=128)
        iw = CH_CLS // 16            # idx columns per chunk
        for t in range(T):
            xt = xp.tile([128, ROWS_P, FEAT], mybir.dt.float32)
            nc.sync.dma_start(xt[:], xr[t])
            ct = cp.tile([128, 1, FEAT], mybir.dt.float32)
            nc.gpsimd.dma_gather(ct[:], tbl.ap(),
                                 idxt[:, t * iw:(t + 1) * iw],
                                 CH_CLS, CH_CLS, FEAT, queue_num=t % 4)
            xb = xbp.tile([128, ROWS_P, FEAT], mybir.dt.bfloat16)
            nc.vector.tensor_copy(xb[:], xt[:])
            st = pp.tile([CH_CLS, FEAT], mybir.dt.float32, space="PSUM")
            for u in range(ROWS_P):
                col = (t * ROWS_P + u) * CH_CLS
                nc.tensor.matmul(st[:],
                                 lhsT=a_sb[:, col:col + CH_CLS],
                                 rhs=xb[:, u, :],
                                 start=(u == 0), stop=(u == ROWS_P - 1))
            # cross term: acc[0:32, T+1-ish] gets -2 * sum_f S.c per class;
            # accumulate per chunk into its own column of csq-like storage.
            # The custom-DVE tensor_tensor_reduce wedges the device on this
            # runtime path (HW-bisected), so stage S through SBUF and use
            # stock mul + reduce instead.
            ss = scp.tile([CH_CLS, FEAT], mybir.dt.float32)
            nc.vector.tensor_copy(ss[:], st[:])
            sc = scp.tile([CH_CLS, FEAT], mybir.dt.float32)
            nc.vector.tensor_mul(sc[:], ss[:], ct[0:CH_CLS, 0, :])
            nc.vector.tensor_reduce(crossc[:, t:t + 1], sc[:],
                                    mybir.AxisListType.X, mybir.AluOpType.add)
            # |c|^2 per window row, squaring ct in place (safe: the DVE
            # read above is sequenced first by the tile framework).
            nc.scalar.activation(ct[0:CH_CLS, 0, :], ct[0:CH_CLS, 0, :],
                                 mybir.ActivationFunctionType.Square,
                                 accum_out=csq[:, t:t + 1])
            # sum x^2 of the chunk (in place), row-accumulated.
            nc.scalar.activation(xt[:], xt[:],
                                 mybir.ActivationFunctionType.Square,
                                 accum_out=acc[:, t:t + 1])

        # Epilogue: counts-weight the per-chunk |c|^2 columns, fold both
        # small terms into acc's last two columns, then one global reduce.
        scr = scp.tile([CH_CLS, T], mybir.dt.float32)
        nc.vector.tensor_mul(scr[:], csq[:], cntt[:])
        nc.vector.tensor_reduce(acc[0:CH_CLS, T:T + 1], scr[:],
                                mybir.AxisListType.X, mybir.AluOpType.add)
        nc.vector.tensor_reduce(acc[0:CH_CLS, T + 1:T + 2], crossc[:],
                                mybir.AxisListType.X, mybir.AluOpType.add)
        nc.vector.tensor_scalar_mul(acc[0:CH_CLS, T + 1:T + 2],
                                    acc[0:CH_CLS, T + 1:T + 2], -2.0)
        ones = sp.tile([128, 1], mybir.dt.float32)
        nc.vector.memset(ones[:], 1.0)
        red = sp.tile([128, 1], mybir.dt.float32)
        nc.vector.tensor_reduce(red[:], acc[:], mybir.AxisListType.X,
                                mybir.AluOpType.add)
        tot = pp.tile([1, 1], mybir.dt.float32, space="PSUM")
        nc.tensor.matmul(tot[:], lhsT=red[:], rhs=ones[:],
                         start=True, stop=True)
        tot_sb = sp.tile([1, 1], mybir.dt.float32)
        nc.vector.tensor_copy(tot_sb[:], tot[:])
        nc.sync.dma_start(out.ap(), tot_sb[:])
    nc.compile()
    return nc


_NC = None


def _get_nc():
    global _NC
    if _NC is None:
        _NC = _build_nc()
    return _NC


def _prep_core(labels_shard):
    """Per-core host prep from the SORTED label shard: one-hot A tiles,
    wrapped int16 gather indices, per-(slot, chunk) counts."""
    A = np.zeros((128, T * ROWS_P * CH_CLS), dtype=ml_dtypes.bfloat16)
    idx16 = np.zeros((16, T * CH_CLS // 16), dtype=np.int16)
    counts = np.zeros((CH_CLS, T), dtype=np.float32)
    p_idx = np.arange(CHUNK) // ROWS_P
    u_idx = np.arange(CHUNK) % ROWS_P
    iw = CH_CLS // 16
    for t in range(T):
        lab = labels_shard[t * CHUNK:(t + 1) * CHUNK]
        classes, cnt = np.unique(lab, return_counts=True)
        k = len(classes)
        assert k <= CH_CLS, f"chunk spans {k} classes > {CH_CLS}"
        win = np.full(CH_CLS, classes[-1], dtype=np.int64)
        win[:k] = classes
        counts[:k, t] = cnt
        slot = np.searchsorted(win[:k], lab)
        A3 = np.zeros((128, ROWS_P, CH_CLS), dtype=np.float32)
        A3[p_idx, u_idx, slot] = 1.0
        A[:, t * ROWS_P * CH_CLS:(t + 1) * ROWS_P * CH_CLS] = \
            A3.reshape(128, ROWS_P * CH_CLS)
        idx16[:, t * iw:(t + 1) * iw] = \
            win.reshape(iw, 16).T.astype(np.int16)
    return A, np.tile(idx16, (8, 1)), counts


def kernel(input_x, input_labels, target_x):
    global LAST_RESULTS
    input_x = np.ascontiguousarray(np.asarray(input_x), dtype=np.float32)
    labels = np.asarray(input_labels).astype(np.int64)
    table = np.ascontiguousarray(np.asarray(target_x), dtype=np.float32)
    assert input_x.shape == (N, FEAT) and labels.shape == (N,)
    assert table.shape == (NCLASS, FEAT)

    perm = np.argsort(labels, kind="stable")
    xs = input_x[perm]
    ls = labels[perm]

    nc = _get_nc()
    in_maps = []
    for c in range(NCORES):
        sl = slice(c * SHARD, (c + 1) * SHARD)
        A, idx16, counts = _prep_core(ls[sl])
        in_maps.append({
            "x": xs[sl],
            "a": A,
            "idxs": idx16,
            "cnts": counts,
            "tbl": table,
        })
    res = run_bass_kernel_spmd(nc, in_maps, list(range(NCORES)), trace=TRACE)
    LAST_RESULTS = res
    partials = [np.float64(r["out"][0, 0]) for r in res.results]
    return np.float32(sum(partials) / (N * FEAT))
